# revision 32
# baseline (speedup 1.0000x reference)
"""Block-sparse linear y = x @ W^T on 8 Trainium2 NeuronCores.

Strategy: the 32x32 block structure (50% block density, random scatter) is not
exploitable on a 128x128 PE array (M=32 tiles run at 25% utilization and the
per-block LDWEIGHTS cost dominates), so we densify W^T on the host (cheap: 8MB
of scatter-adds) and run a dense GEMM, sharded 4-way over tokens x 2-way over
out_features (8 cores, no collectives). That sharding minimizes per-core HBM
traffic (20MB: x^T 8 + W^T 8 + y 4) — the kernel sits right at the ridge of
the DMA (~20MB / ~360GB/s) and PE (256 matmuls x ~227ns) rooflines.

Matmuls run in float32r (single-pass fp32 mode, full PE rate at N=512,
~1.2e-4 scale-relative error vs the fp32 reference). MM_DTYPE="float32"
switches to exact 2-pass fp32 at 4x the PE cost.

Schedule per core, three phases: (1) out-half 0 over the first K-half,
k-outer with x and W staged just-in-time (this is the only window where
fresh x + W demand exceeds DMA supply); psums evict to SBUF partials.
(2) out-half 1 over the FULL K in one psum accumulation (no partials
needed), k-outer, with the second-half x supertiles slotted into the W
stream's bandwidth slack. (3) out-half 0 over the second K-half, k-inner
per psum bank so each bank adds its partial and drains the moment it
finishes. All inputs are pre-blocked on the host into exact SBUF layouts so
every load is a linear DMA with up-to-16KB contiguous runs; the data stream
(x + W) rides the sync queue in consumption-priority order while outputs go
on the scalar queue, so neither stream's waits can block the other. A few
dummy matmuls at the start warm the PE clock gate during the DMA head wait.
"""

import numpy as np

TOKENS, IN_F, OUT_F = 4096, 2048, 2048
BLOCK = 32
N_CORES = 8
TG, OG = 4, 2  # token groups x out-feature groups
T_SH = TOKENS // TG  # 1024 tokens per core
O_SH = OUT_F // OG  # 1024 out features per core
P = 128
NFREE = 512  # PSUM bank free dim (fp32)
KT = IN_F // P  # 16 k tiles
MT = T_SH // P  # 8 psum banks
NT = O_SH // NFREE  # 2 out column tiles
KH = KT // 2  # 8 k-tiles per K-half
XH = T_SH // 2  # token half
QUARTERS = [(0, 0), (1, 0), (0, 1), (1, 1)]  # (out-half n, K-half kh)

MM_DTYPE = "float32r"  # "float32r" (fast) or "float32" (exact 2-pass)
TRACE = False  # set by test.py to capture an NTFF profile

_nc_cache = {}
_last_result = None  # BassKernelResults of the most recent run (for test.py)


def _build_nc():
    import concourse.mybir as mybir
    import concourse.tile as tile
    from concourse import bacc

    key = MM_DTYPE
    if key in _nc_cache:
        return _nc_cache[key]

    dt_mm = getattr(mybir.dt, MM_DTYPE)
    f32 = mybir.dt.float32

    nc = bacc.Bacc(None, target_bir_lowering=False)
    # Host-pre-blocked inputs (exact SBUF layouts; all DMAs are linear):
    # xa: A-half x^T k-tiles, [KH][P][T_SH]
    # xb: B-half x^T supertiles by token-half, [2][P][KH][XH]
    # wq: per-quarter W^T supertiles, [4][P][KH][NFREE]
    xa = nc.dram_tensor("xa", [KH, P, T_SH], dt_mm, kind="ExternalInput")
    xb = nc.dram_tensor("xb", [2, P, KH, XH], dt_mm, kind="ExternalInput")
    wq4 = nc.dram_tensor("wq", [4, P, KH, NFREE], dt_mm, kind="ExternalInput")
    y = nc.dram_tensor("y", [T_SH, O_SH], f32, kind="ExternalOutput")

    with tile.TileContext(nc) as tc:
        with (
            tc.tile_pool(name="xp", bufs=1) as xp,
            tc.tile_pool(name="wp", bufs=1) as wp,
            tc.tile_pool(name="pp", bufs=1) as pp,
            tc.tile_pool(name="op", bufs=4) as op,
            tc.tile_pool(name="ps", bufs=1, space="PSUM") as ps,
        ):
            NSPLIT = 4  # first k-tiles split in token-halves for a fast ramp
            xa_t = [None] * KH  # [P, T_SH] tiles (ki < NSPLIT: two halves)
            xah = [[None, None] for _ in range(NSPLIT)]
            xbs = [None, None]  # [P, KH, XH] supertiles

            # Warm the PE's HAM clock gate during the initial DMA wait: ~6
            # dummy matmuls on a zeroed tile take the array past the 3.4us
            # busy window, so the first real matmuls run at 2.4GHz.
            zt = xp.tile([P, NFREE], f32, tag="warm", name="warm")
            nc.gpsimd.memset(zt[:], 0.0)
            warm_ps = ps.tile([P, NFREE], f32, tag="ps0", name="warm_ps")
            for _ in range(3):
                nc.tensor.matmul(warm_ps[:], zt[:, :P], zt[:], start=True, stop=True)

            def lhsT(m, k):
                """Stationary x^T slice for bank m, global k-tile k."""
                if k >= KH:
                    return xbs[m // 4][:, k - KH, (m % 4) * P : (m % 4 + 1) * P]
                if k < NSPLIT:
                    return xah[k][m // 4][:, (m % 4) * P : (m % 4 + 1) * P]
                return xa_t[k][:, m * P : (m + 1) * P]

            def fresh_psums():
                return [
                    ps.tile([P, NFREE], f32, tag=f"ps{m}", name=f"ps{m}")
                    for m in range(MT)
                ]

            # ---- Phase 1: (n=0, K first half), k-outer, x + W staged JIT ----
            psums = fresh_psums()
            w0 = []
            for ki in range(KH):

                def load_w0():
                    wt = wp.tile([P, NFREE], dt_mm, tag=f"w0_{ki}", name=f"w0_{ki}")
                    nc.sync.dma_start(wt[:], wq4[0, :, ki, :])
                    w0.append(wt)

                if ki < NSPLIT:  # token-halves with W between: fast ramp
                    for h in range(2):
                        t = xp.tile(
                            [P, XH], dt_mm, tag=f"xa{ki}_{h}", name=f"xa{ki}_{h}"
                        )
                        nc.sync.dma_start(t[:], xa[ki, :, h * XH : (h + 1) * XH])
                        xah[ki][h] = t
                        if h == 0:
                            load_w0()
                else:
                    t = xp.tile([P, T_SH], dt_mm, tag=f"xa{ki}", name=f"xa{ki}")
                    nc.sync.dma_start(t[:], xa[ki])
                    xa_t[ki] = t
                    load_w0()
                for m in range(MT):
                    nc.tensor.matmul(
                        psums[m][:],
                        lhsT(m, ki),
                        w0[ki][:],
                        start=(ki == 0),
                        stop=(ki == KH - 1),
                    )
            partials = []
            for m in range(MT):  # evict n=0 partial sums to SBUF
                pt = pp.tile([P, NFREE], f32, tag=f"pt{m}", name=f"pt{m}")
                nc.vector.tensor_copy(pt[:], psums[m][:])
                partials.append(pt)

            # ---- Phase 2: (n=1, FULL K), k-outer; single psum accumulation
            # so n=1 needs no partials. W per-k JIT with the B-half x
            # supertiles slotted into the stream's slack. ----
            psums = fresh_psums()
            w1 = []
            for k in range(KT):
                # B-half x, token-half 0 (first used at k=8, m=0): loaded in
                # four 512KB chunks slotted between W tiles, so the big
                # transfer never delays the next W tile in the queue.
                if k in (1, 3, 5, 6):
                    ci = {1: 0, 3: 1, 5: 2, 6: 3}[k]
                    if ci == 0:
                        xbs[0] = xp.tile([P, KH, XH], dt_mm, tag="xb0", name="xb0")
                    nc.sync.dma_start(
                        xbs[0][:, 2 * ci : 2 * ci + 2, :], xb[0, :, 2 * ci : 2 * ci + 2, :]
                    )
                if k == 8:  # token-half 1 (first used at k=8, m=4)
                    xbs[1] = xp.tile([P, KH, XH], dt_mm, tag="xb1", name="xb1")
                    nc.sync.dma_start(xbs[1][:], xb[1])
                wt = wp.tile([P, NFREE], dt_mm, tag=f"w1_{k}", name=f"w1_{k}")
                src = wq4[1, :, k, :] if k < KH else wq4[3, :, k - KH, :]
                nc.sync.dma_start(wt[:], src)
                w1.append(wt)
                for m in range(MT):
                    nc.tensor.matmul(
                        psums[m][:],
                        lhsT(m, k),
                        w1[k][:],
                        start=(k == 0),
                        stop=(k == KT - 1),
                    )
            for m in range(MT):  # n=1 done: copy out directly
                ot = op.tile([P, NFREE], f32, tag="ot")
                nc.vector.tensor_copy(ot[:], psums[m][:])
                nc.scalar.dma_start(
                    y[m * P : (m + 1) * P, NFREE : 2 * NFREE], ot[:]
                )

            # ---- Phase 3: (n=0, K second half), k-inner per bank so each
            # bank adds its partial and drains as soon as it finishes. ----
            w3 = wp.tile([P, KH, NFREE], dt_mm, tag="w3", name="w3")
            nc.sync.dma_start(w3[:], wq4[2])
            psums = fresh_psums()
            for m in range(MT):
                for ki in range(KH):
                    nc.tensor.matmul(
                        psums[m][:],
                        lhsT(m, KH + ki),
                        w3[:, ki, :],
                        start=(ki == 0),
                        stop=(ki == KH - 1),
                    )
                ot = op.tile([P, NFREE], f32, tag="ot")
                nc.vector.tensor_add(
                    out=ot[:], in0=psums[m][:], in1=partials[m][:]
                )
                nc.scalar.dma_start(y[m * P : (m + 1) * P, 0:NFREE], ot[:])

    nc.compile()
    _nc_cache[key] = nc
    return nc


def _densify_wT(weight_blocks, block_rows, block_cols):
    """Scatter-add the 32x32 blocks into dense W^T [in_features, out_features]."""
    nc_blk = IN_F // BLOCK
    nr_blk = OUT_F // BLOCK
    wcr = np.zeros((nc_blk, nr_blk, BLOCK, BLOCK), np.float32)
    # block b occupies W[32r:32r+32, 32c:32c+32]; W^T gets the transposed block
    np.add.at(
        wcr,
        (block_cols.astype(np.int64), block_rows.astype(np.int64)),
        np.swapaxes(weight_blocks.astype(np.float32, copy=False), 1, 2),
    )
    return np.ascontiguousarray(wcr.transpose(0, 2, 1, 3).reshape(IN_F, OUT_F))


def _pack_core_inputs(xT_sh, wT_sh):
    """Block one core's x^T and W^T shards into the kernel's DMA layouts."""
    Xsh = xT_sh.reshape(KT, P, T_SH)
    xa = np.ascontiguousarray(Xsh[:KH])  # [KH, P, T_SH]
    xb = np.ascontiguousarray(  # [2, P, KH, XH]
        np.stack(
            [
                Xsh[KH:, :, :XH].transpose(1, 0, 2),
                Xsh[KH:, :, XH:].transpose(1, 0, 2),
            ]
        )
    )
    Wsh = wT_sh.reshape(2, KH, P, O_SH)  # [kh, ki, p, f]
    wq = np.ascontiguousarray(  # [4, P, KH, NFREE], quarter order
        np.stack(
            [
                Wsh[kh, :, :, n * NFREE : (n + 1) * NFREE].transpose(1, 0, 2)
                for (n, kh) in QUARTERS
            ]
        )
    )
    return {"xa": xa, "xb": xb, "wq": wq}


def kernel(x, weight_blocks, block_rows, block_cols):
    global _last_result
    from concourse.bass_utils import run_bass_kernel_spmd

    x = np.asarray(x, dtype=np.float32)
    wT = _densify_wT(
        np.asarray(weight_blocks), np.asarray(block_rows), np.asarray(block_cols)
    )
    xT = np.ascontiguousarray(x.T)

    in_maps = []
    for c in range(N_CORES):
        tg, og = divmod(c, OG)
        in_maps.append(
            _pack_core_inputs(
                xT[:, tg * T_SH : (tg + 1) * T_SH],
                wT[:, og * O_SH : (og + 1) * O_SH],
            )
        )

    nc = _build_nc()
    res = None
    for attempt in range(3):  # transient NRT device errors happen; retry
        try:
            res = run_bass_kernel_spmd(
                nc, in_maps, core_ids=list(range(N_CORES)), trace=TRACE
            )
            break
        except Exception:
            if attempt == 2:
                raise
            import time

            time.sleep(3)
    _last_result = res

    y = np.empty((TOKENS, OUT_F), np.float32)
    for c in range(N_CORES):
        tg, og = divmod(c, OG)
        y[tg * T_SH : (tg + 1) * T_SH, og * O_SH : (og + 1) * O_SH] = res.results[c][
            "y"
        ]
    return y


# revision 33
# speedup vs baseline: 1.0451x; 1.0451x over previous
"""Block-sparse linear y = x @ W^T on 8 Trainium2 NeuronCores.

Strategy: the 32x32 block structure (50% block density, random scatter) is not
exploitable on a 128x128 PE array (M=32 tiles run at 25% utilization and the
per-block LDWEIGHTS cost dominates), so we densify W^T on the host (cheap: 8MB
of scatter-adds) and run a dense GEMM, sharded 4-way over tokens x 2-way over
out_features (8 cores, no collectives). That sharding minimizes per-core HBM
traffic (20MB: x^T 8 + W^T 8 + y 4) — the kernel sits right at the ridge of
the DMA (~20MB / ~360GB/s) and PE (256 matmuls x ~227ns) rooflines.

Matmuls run in float32r (single-pass fp32 mode, full PE rate at N=512,
~1.2e-4 scale-relative error vs the fp32 reference). MM_DTYPE="float32"
switches to exact 2-pass fp32 at 4x the PE cost.

Schedule per core, three phases: (1) out-half 0 over the first K-half,
k-outer with x and W staged just-in-time (this is the only window where
fresh x + W demand exceeds DMA supply); psums evict to SBUF partials.
(2) out-half 1 over the FULL K in one psum accumulation (no partials
needed), k-outer, with the second-half x supertiles slotted into the W
stream's bandwidth slack. (3) out-half 0 over the second K-half, k-inner
per psum bank so each bank adds its partial and drains the moment it
finishes. All inputs are pre-blocked on the host into exact SBUF layouts so
every load is a linear DMA with up-to-16KB contiguous runs; the data stream
(x + W) rides the sync queue in consumption-priority order while outputs go
on the scalar queue, so neither stream's waits can block the other. A few
dummy matmuls at the start warm the PE clock gate during the DMA head wait.
"""

import numpy as np

TOKENS, IN_F, OUT_F = 4096, 2048, 2048
BLOCK = 32
N_CORES = 8
TG, OG = 4, 2  # token groups x out-feature groups
T_SH = TOKENS // TG  # 1024 tokens per core
O_SH = OUT_F // OG  # 1024 out features per core
P = 128
NFREE = 512  # PSUM bank free dim (fp32)
KT = IN_F // P  # 16 k tiles
MT = T_SH // P  # 8 psum banks
NT = O_SH // NFREE  # 2 out column tiles
KH = KT // 2  # 8 k-tiles per K-half
XH = T_SH // 2  # token half
QUARTERS = [(0, 0), (1, 0), (0, 1), (1, 1)]  # (out-half n, K-half kh)

MM_DTYPE = "float32r"  # "float32r" (fast) or "float32" (exact 2-pass)
TRACE = False  # set by test.py to capture an NTFF profile

_nc_cache = {}
_last_result = None  # BassKernelResults of the most recent run (for test.py)


def _build_nc():
    import concourse.mybir as mybir
    import concourse.tile as tile
    from concourse import bacc

    key = MM_DTYPE
    if key in _nc_cache:
        return _nc_cache[key]

    dt_mm = getattr(mybir.dt, MM_DTYPE)
    f32 = mybir.dt.float32

    nc = bacc.Bacc(None, target_bir_lowering=False)
    # Host-pre-blocked inputs (exact SBUF layouts; all DMAs are linear):
    # xa: A-half x^T k-tiles, [KH][P][T_SH]
    # xb: B-half x^T supertiles by token-half, [2][P][KH][XH]
    # wq: per-quarter W^T supertiles, [4][P][KH][NFREE]
    xa = nc.dram_tensor("xa", [KH, P, T_SH], dt_mm, kind="ExternalInput")
    xb = nc.dram_tensor("xb", [2, P, KH, XH], dt_mm, kind="ExternalInput")
    wq4 = nc.dram_tensor("wq", [4, P, KH, NFREE], dt_mm, kind="ExternalInput")
    y = nc.dram_tensor("y", [T_SH, O_SH], f32, kind="ExternalOutput")

    with tile.TileContext(nc) as tc:
        with (
            tc.tile_pool(name="xp", bufs=1) as xp,
            tc.tile_pool(name="wp", bufs=1) as wp,
            tc.tile_pool(name="pp", bufs=1) as pp,
            tc.tile_pool(name="op", bufs=4) as op,
            tc.tile_pool(name="ps", bufs=1, space="PSUM") as ps,
        ):
            NSPLIT = 4  # first k-tiles split in token-halves for a fast ramp
            xa_t = [None] * KH  # [P, T_SH] tiles (ki < NSPLIT: two halves)
            xah = [[None, None] for _ in range(NSPLIT)]
            xbs = [None, None]  # [P, KH, XH] supertiles

            # Warm the PE's HAM clock gate during the initial DMA wait: ~6
            # dummy matmuls on a zeroed tile take the array past the 3.4us
            # busy window, so the first real matmuls run at 2.4GHz.
            zt = xp.tile([P, NFREE], f32, tag="warm", name="warm")
            nc.gpsimd.memset(zt[:], 0.0)
            warm_ps = ps.tile([P, NFREE], f32, tag="ps0", name="warm_ps")
            for _ in range(3):
                nc.tensor.matmul(warm_ps[:], zt[:, :P], zt[:], start=True, stop=True)

            def lhsT(m, k):
                """Stationary x^T slice for bank m, global k-tile k."""
                if k >= KH:
                    return xbs[m // 4][:, k - KH, (m % 4) * P : (m % 4 + 1) * P]
                if k < NSPLIT:
                    return xah[k][m // 4][:, (m % 4) * P : (m % 4 + 1) * P]
                return xa_t[k][:, m * P : (m + 1) * P]

            def fresh_psums():
                return [
                    ps.tile([P, NFREE], f32, tag=f"ps{m}", name=f"ps{m}")
                    for m in range(MT)
                ]

            # ---- Phase 1: (n=0, K first half), k-outer, x + W staged JIT ----
            psums = fresh_psums()
            w0 = []
            for ki in range(KH):

                def load_w0():
                    wt = wp.tile([P, NFREE], dt_mm, tag=f"w0_{ki}", name=f"w0_{ki}")
                    nc.sync.dma_start(wt[:], wq4[0, :, ki, :])
                    w0.append(wt)

                if ki < NSPLIT:  # token-halves with W between: fast ramp
                    for h in range(2):
                        t = xp.tile(
                            [P, XH], dt_mm, tag=f"xa{ki}_{h}", name=f"xa{ki}_{h}"
                        )
                        nc.sync.dma_start(t[:], xa[ki, :, h * XH : (h + 1) * XH])
                        xah[ki][h] = t
                        if h == 0:
                            load_w0()
                else:
                    t = xp.tile([P, T_SH], dt_mm, tag=f"xa{ki}", name=f"xa{ki}")
                    nc.sync.dma_start(t[:], xa[ki])
                    xa_t[ki] = t
                    load_w0()
                for m in range(MT):
                    nc.tensor.matmul(
                        psums[m][:],
                        lhsT(m, ki),
                        w0[ki][:],
                        start=(ki == 0),
                        stop=(ki == KH - 1),
                    )
            partials = []
            for m in range(MT):  # evict n=0 partial sums to SBUF
                pt = pp.tile([P, NFREE], f32, tag=f"pt{m}", name=f"pt{m}")
                nc.vector.tensor_copy(pt[:], psums[m][:])
                partials.append(pt)

            # ---- Phase 2: (n=1, FULL K), k-outer; single psum accumulation
            # so n=1 needs no partials. W per-k JIT with the B-half x
            # supertiles slotted into the stream's slack. ----
            psums = fresh_psums()
            w1 = []
            for k in range(KT):
                # B-half x, token-half 0 (first used at k=8, m=0): loaded in
                # four 512KB chunks slotted between W tiles, so the big
                # transfer never delays the next W tile in the queue.
                if k in (1, 3, 5, 6):
                    ci = {1: 0, 3: 1, 5: 2, 6: 3}[k]
                    if ci == 0:
                        xbs[0] = xp.tile([P, KH, XH], dt_mm, tag="xb0", name="xb0")
                    nc.sync.dma_start(
                        xbs[0][:, 2 * ci : 2 * ci + 2, :], xb[0, :, 2 * ci : 2 * ci + 2, :]
                    )
                if k == 8:  # token-half 1 (first used at k=8, m=4)
                    xbs[1] = xp.tile([P, KH, XH], dt_mm, tag="xb1", name="xb1")
                    nc.sync.dma_start(xbs[1][:], xb[1])
                wt = wp.tile([P, NFREE], dt_mm, tag=f"w1_{k}", name=f"w1_{k}")
                src = wq4[1, :, k, :] if k < KH else wq4[3, :, k - KH, :]
                nc.sync.dma_start(wt[:], src)
                w1.append(wt)
                for m in range(MT):
                    nc.tensor.matmul(
                        psums[m][:],
                        lhsT(m, k),
                        w1[k][:],
                        start=(k == 0),
                        stop=(k == KT - 1),
                    )
            for m in range(MT):  # n=1 done: copy out directly
                ot = op.tile([P, NFREE], f32, tag="ot")
                nc.vector.tensor_copy(ot[:], psums[m][:])
                nc.scalar.dma_start(
                    y[m * P : (m + 1) * P, NFREE : 2 * NFREE], ot[:]
                )

            # ---- Phase 3: (n=0, K second half), k-inner per bank so each
            # bank adds its partial and drains as soon as it finishes. ----
            w3 = wp.tile([P, KH, NFREE], dt_mm, tag="w3", name="w3")
            nc.sync.dma_start(w3[:], wq4[2])
            psums = fresh_psums()
            for m in range(MT):
                for ki in range(KH):
                    nc.tensor.matmul(
                        psums[m][:],
                        lhsT(m, KH + ki),
                        w3[:, ki, :],
                        start=(ki == 0),
                        stop=(ki == KH - 1),
                    )
                ot = op.tile([P, NFREE], f32, tag="ot")
                if m == MT - 1:
                    # last bank is on the critical path to the kernel-end
                    # drain: halve its add+store so the first half's DMA
                    # overlaps the second half's add.
                    NH = NFREE // 2
                    for hh in range(2):
                        nc.vector.tensor_add(
                            out=ot[:, hh * NH : (hh + 1) * NH],
                            in0=psums[m][:, hh * NH : (hh + 1) * NH],
                            in1=partials[m][:, hh * NH : (hh + 1) * NH],
                        )
                        nc.scalar.dma_start(
                            y[m * P : (m + 1) * P, hh * NH : (hh + 1) * NH],
                            ot[:, hh * NH : (hh + 1) * NH],
                        )
                else:
                    nc.vector.tensor_add(
                        out=ot[:], in0=psums[m][:], in1=partials[m][:]
                    )
                    nc.scalar.dma_start(y[m * P : (m + 1) * P, 0:NFREE], ot[:])

    nc.compile()
    _nc_cache[key] = nc
    return nc


def _densify_wT(weight_blocks, block_rows, block_cols):
    """Scatter-add the 32x32 blocks into dense W^T [in_features, out_features]."""
    nc_blk = IN_F // BLOCK
    nr_blk = OUT_F // BLOCK
    wcr = np.zeros((nc_blk, nr_blk, BLOCK, BLOCK), np.float32)
    # block b occupies W[32r:32r+32, 32c:32c+32]; W^T gets the transposed block
    np.add.at(
        wcr,
        (block_cols.astype(np.int64), block_rows.astype(np.int64)),
        np.swapaxes(weight_blocks.astype(np.float32, copy=False), 1, 2),
    )
    return np.ascontiguousarray(wcr.transpose(0, 2, 1, 3).reshape(IN_F, OUT_F))


def _pack_core_inputs(xT_sh, wT_sh):
    """Block one core's x^T and W^T shards into the kernel's DMA layouts."""
    Xsh = xT_sh.reshape(KT, P, T_SH)
    xa = np.ascontiguousarray(Xsh[:KH])  # [KH, P, T_SH]
    xb = np.ascontiguousarray(  # [2, P, KH, XH]
        np.stack(
            [
                Xsh[KH:, :, :XH].transpose(1, 0, 2),
                Xsh[KH:, :, XH:].transpose(1, 0, 2),
            ]
        )
    )
    Wsh = wT_sh.reshape(2, KH, P, O_SH)  # [kh, ki, p, f]
    wq = np.ascontiguousarray(  # [4, P, KH, NFREE], quarter order
        np.stack(
            [
                Wsh[kh, :, :, n * NFREE : (n + 1) * NFREE].transpose(1, 0, 2)
                for (n, kh) in QUARTERS
            ]
        )
    )
    return {"xa": xa, "xb": xb, "wq": wq}


def kernel(x, weight_blocks, block_rows, block_cols):
    global _last_result
    from concourse.bass_utils import run_bass_kernel_spmd

    x = np.asarray(x, dtype=np.float32)
    wT = _densify_wT(
        np.asarray(weight_blocks), np.asarray(block_rows), np.asarray(block_cols)
    )
    xT = np.ascontiguousarray(x.T)

    in_maps = []
    for c in range(N_CORES):
        tg, og = divmod(c, OG)
        in_maps.append(
            _pack_core_inputs(
                xT[:, tg * T_SH : (tg + 1) * T_SH],
                wT[:, og * O_SH : (og + 1) * O_SH],
            )
        )

    nc = _build_nc()
    res = None
    for attempt in range(3):  # transient NRT device errors happen; retry
        try:
            res = run_bass_kernel_spmd(
                nc, in_maps, core_ids=list(range(N_CORES)), trace=TRACE
            )
            break
        except Exception:
            if attempt == 2:
                raise
            import time

            time.sleep(3)
    _last_result = res

    y = np.empty((TOKENS, OUT_F), np.float32)
    for c in range(N_CORES):
        tg, og = divmod(c, OG)
        y[tg * T_SH : (tg + 1) * T_SH, og * O_SH : (og + 1) * O_SH] = res.results[c][
            "y"
        ]
    return y


# revision 34
# speedup vs baseline: 1.0502x; 1.0049x over previous
"""Block-sparse linear y = x @ W^T on 8 Trainium2 NeuronCores.

Strategy: the 32x32 block structure (50% block density, random scatter) is not
exploitable on a 128x128 PE array (M=32 tiles run at 25% utilization and the
per-block LDWEIGHTS cost dominates), so we densify W^T on the host (cheap: 8MB
of scatter-adds) and run a dense GEMM, sharded 4-way over tokens x 2-way over
out_features (8 cores, no collectives). That sharding minimizes per-core HBM
traffic (20MB: x^T 8 + W^T 8 + y 4) — the kernel sits right at the ridge of
the DMA (~20MB / ~360GB/s) and PE (256 matmuls x ~227ns) rooflines.

Matmuls run in float32r (single-pass fp32 mode, full PE rate at N=512,
~1.2e-4 scale-relative error vs the fp32 reference). MM_DTYPE="float32"
switches to exact 2-pass fp32 at 4x the PE cost.

Schedule per core, three phases: (1) out-half 0 over the first K-half,
k-outer with x and W staged just-in-time (this is the only window where
fresh x + W demand exceeds DMA supply); psums evict to SBUF partials.
(2) out-half 1 over the FULL K in one psum accumulation (no partials
needed), k-outer, with the second-half x supertiles slotted into the W
stream's bandwidth slack. (3) out-half 0 over the second K-half, k-inner
per psum bank so each bank adds its partial and drains the moment it
finishes. All inputs are pre-blocked on the host into exact SBUF layouts so
every load is a linear DMA with up-to-16KB contiguous runs; the data stream
(x + W) rides the sync queue in consumption-priority order while outputs go
on the scalar queue, so neither stream's waits can block the other. A few
dummy matmuls at the start warm the PE clock gate during the DMA head wait.
"""

import numpy as np

TOKENS, IN_F, OUT_F = 4096, 2048, 2048
BLOCK = 32
N_CORES = 8
TG, OG = 4, 2  # token groups x out-feature groups
T_SH = TOKENS // TG  # 1024 tokens per core
O_SH = OUT_F // OG  # 1024 out features per core
P = 128
NFREE = 512  # PSUM bank free dim (fp32)
KT = IN_F // P  # 16 k tiles
MT = T_SH // P  # 8 psum banks
NT = O_SH // NFREE  # 2 out column tiles
KH = KT // 2  # 8 k-tiles per K-half
XH = T_SH // 2  # token half
QUARTERS = [(0, 0), (1, 0), (0, 1), (1, 1)]  # (out-half n, K-half kh)

MM_DTYPE = "float32r"  # "float32r" (fast) or "float32" (exact 2-pass)
TRACE = False  # set by test.py to capture an NTFF profile

_nc_cache = {}
_last_result = None  # BassKernelResults of the most recent run (for test.py)


def _build_nc():
    import concourse.mybir as mybir
    import concourse.tile as tile
    from concourse import bacc

    key = MM_DTYPE
    if key in _nc_cache:
        return _nc_cache[key]

    dt_mm = getattr(mybir.dt, MM_DTYPE)
    f32 = mybir.dt.float32

    nc = bacc.Bacc(None, target_bir_lowering=False)
    # Host-pre-blocked inputs (exact SBUF layouts; all DMAs are linear):
    # xa: A-half x^T k-tiles, [KH][P][T_SH]
    # xb: B-half x^T supertiles by token-half, [2][P][KH][XH]
    # wq: per-quarter W^T supertiles, [4][P][KH][NFREE]
    xa = nc.dram_tensor("xa", [KH, P, T_SH], dt_mm, kind="ExternalInput")
    xb = nc.dram_tensor("xb", [2, P, KH, XH], dt_mm, kind="ExternalInput")
    wq4 = nc.dram_tensor("wq", [4, P, KH, NFREE], dt_mm, kind="ExternalInput")
    y = nc.dram_tensor("y", [T_SH, O_SH], f32, kind="ExternalOutput")

    with tile.TileContext(nc) as tc:
        with (
            tc.tile_pool(name="xp", bufs=1) as xp,
            tc.tile_pool(name="wp", bufs=1) as wp,
            tc.tile_pool(name="pp", bufs=1) as pp,
            tc.tile_pool(name="op", bufs=4) as op,
            tc.tile_pool(name="ps", bufs=1, space="PSUM") as ps,
        ):
            NSPLIT = 4  # first k-tiles split in token-halves for a fast ramp
            xa_t = [None] * KH  # [P, T_SH] tiles (ki < NSPLIT: two halves)
            xah = [[None, None] for _ in range(NSPLIT)]
            xbs = [None, None]  # [P, KH, XH] supertiles

            # Warm the PE's HAM clock gate during the initial DMA wait: ~6
            # dummy matmuls on a zeroed tile take the array past the 3.4us
            # busy window, so the first real matmuls run at 2.4GHz.
            zt = xp.tile([P, NFREE], f32, tag="warm", name="warm")
            nc.gpsimd.memset(zt[:], 0.0)
            warm_ps = ps.tile([P, NFREE], f32, tag="ps0", name="warm_ps")
            for _ in range(3):
                nc.tensor.matmul(warm_ps[:], zt[:, :P], zt[:], start=True, stop=True)

            def lhsT(m, k):
                """Stationary x^T slice for bank m, global k-tile k."""
                if k >= KH:
                    return xbs[m // 4][:, k - KH, (m % 4) * P : (m % 4 + 1) * P]
                if k < NSPLIT:
                    return xah[k][m // 4][:, (m % 4) * P : (m % 4 + 1) * P]
                return xa_t[k][:, m * P : (m + 1) * P]

            def fresh_psums():
                return [
                    ps.tile([P, NFREE], f32, tag=f"ps{m}", name=f"ps{m}")
                    for m in range(MT)
                ]

            # ---- Phase 1: (n=0, K first half), k-outer, x + W staged JIT ----
            psums = fresh_psums()
            w0 = []
            for ki in range(KH):

                def load_w0():
                    wt = wp.tile([P, NFREE], dt_mm, tag=f"w0_{ki}", name=f"w0_{ki}")
                    nc.sync.dma_start(wt[:], wq4[0, :, ki, :])
                    w0.append(wt)

                if ki < NSPLIT:  # token-halves with W between: fast ramp
                    for h in range(2):
                        t = xp.tile(
                            [P, XH], dt_mm, tag=f"xa{ki}_{h}", name=f"xa{ki}_{h}"
                        )
                        nc.sync.dma_start(t[:], xa[ki, :, h * XH : (h + 1) * XH])
                        xah[ki][h] = t
                        if h == 0:
                            load_w0()
                else:
                    t = xp.tile([P, T_SH], dt_mm, tag=f"xa{ki}", name=f"xa{ki}")
                    nc.sync.dma_start(t[:], xa[ki])
                    xa_t[ki] = t
                    load_w0()
                for m in range(MT):
                    nc.tensor.matmul(
                        psums[m][:],
                        lhsT(m, ki),
                        w0[ki][:],
                        start=(ki == 0),
                        stop=(ki == KH - 1),
                    )
            partials = []
            for m in range(MT):  # evict n=0 partial sums to SBUF
                pt = pp.tile([P, NFREE], f32, tag=f"pt{m}", name=f"pt{m}")
                nc.vector.tensor_copy(pt[:], psums[m][:])
                partials.append(pt)

            # ---- Phase 2: (n=1, FULL K), k-outer; single psum accumulation
            # so n=1 needs no partials. W per-k JIT with the B-half x
            # supertiles slotted into the stream's slack. ----
            psums = fresh_psums()
            w1 = []
            for k in range(KT):
                # B-half x, token-half 0 (first used at k=8, m=0): loaded in
                # four 512KB chunks slotted between W tiles, so the big
                # transfer never delays the next W tile in the queue.
                if k in (1, 3, 5, 6):
                    ci = {1: 0, 3: 1, 5: 2, 6: 3}[k]
                    if ci == 0:
                        xbs[0] = xp.tile([P, KH, XH], dt_mm, tag="xb0", name="xb0")
                    nc.sync.dma_start(
                        xbs[0][:, 2 * ci : 2 * ci + 2, :], xb[0, :, 2 * ci : 2 * ci + 2, :]
                    )
                if k == 8:  # token-half 1 (first used at k=8, m=4)
                    xbs[1] = xp.tile([P, KH, XH], dt_mm, tag="xb1", name="xb1")
                    nc.sync.dma_start(xbs[1][:], xb[1])
                wt = wp.tile([P, NFREE], dt_mm, tag=f"w1_{k}", name=f"w1_{k}")
                src = wq4[1, :, k, :] if k < KH else wq4[3, :, k - KH, :]
                nc.sync.dma_start(wt[:], src)
                w1.append(wt)
                for m in range(MT):
                    nc.tensor.matmul(
                        psums[m][:],
                        lhsT(m, k),
                        w1[k][:],
                        start=(k == 0),
                        stop=(k == KT - 1),
                    )
            for m in range(MT):  # n=1 done: copy out directly
                ot = op.tile([P, NFREE], f32, tag="ot")
                nc.vector.tensor_copy(ot[:], psums[m][:])
                nc.scalar.dma_start(
                    y[m * P : (m + 1) * P, NFREE : 2 * NFREE], ot[:]
                )

            # ---- Phase 3: (n=0, K second half), k-inner per bank so each
            # bank adds its partial and drains as soon as it finishes. ----
            w3 = wp.tile([P, KH, NFREE], dt_mm, tag="w3", name="w3")
            nc.sync.dma_start(w3[:], wq4[2])
            psums = fresh_psums()
            for m in range(MT):
                for ki in range(KH):
                    nc.tensor.matmul(
                        psums[m][:],
                        lhsT(m, KH + ki),
                        w3[:, ki, :],
                        start=(ki == 0),
                        stop=(ki == KH - 1),
                    )
                ot = op.tile([P, NFREE], f32, tag="ot")
                nc.vector.tensor_add(
                    out=ot[:], in0=psums[m][:], in1=partials[m][:]
                )
                nc.scalar.dma_start(y[m * P : (m + 1) * P, 0:NFREE], ot[:])

    nc.compile()
    _nc_cache[key] = nc
    return nc


def _densify_wT(weight_blocks, block_rows, block_cols):
    """Scatter-add the 32x32 blocks into dense W^T [in_features, out_features]."""
    nc_blk = IN_F // BLOCK
    nr_blk = OUT_F // BLOCK
    wcr = np.zeros((nc_blk, nr_blk, BLOCK, BLOCK), np.float32)
    # block b occupies W[32r:32r+32, 32c:32c+32]; W^T gets the transposed block
    np.add.at(
        wcr,
        (block_cols.astype(np.int64), block_rows.astype(np.int64)),
        np.swapaxes(weight_blocks.astype(np.float32, copy=False), 1, 2),
    )
    return np.ascontiguousarray(wcr.transpose(0, 2, 1, 3).reshape(IN_F, OUT_F))


def _pack_core_inputs(xT_sh, wT_sh):
    """Block one core's x^T and W^T shards into the kernel's DMA layouts."""
    Xsh = xT_sh.reshape(KT, P, T_SH)
    xa = np.ascontiguousarray(Xsh[:KH])  # [KH, P, T_SH]
    xb = np.ascontiguousarray(  # [2, P, KH, XH]
        np.stack(
            [
                Xsh[KH:, :, :XH].transpose(1, 0, 2),
                Xsh[KH:, :, XH:].transpose(1, 0, 2),
            ]
        )
    )
    Wsh = wT_sh.reshape(2, KH, P, O_SH)  # [kh, ki, p, f]
    wq = np.ascontiguousarray(  # [4, P, KH, NFREE], quarter order
        np.stack(
            [
                Wsh[kh, :, :, n * NFREE : (n + 1) * NFREE].transpose(1, 0, 2)
                for (n, kh) in QUARTERS
            ]
        )
    )
    return {"xa": xa, "xb": xb, "wq": wq}


def kernel(x, weight_blocks, block_rows, block_cols):
    global _last_result
    from concourse.bass_utils import run_bass_kernel_spmd

    x = np.asarray(x, dtype=np.float32)
    wT = _densify_wT(
        np.asarray(weight_blocks), np.asarray(block_rows), np.asarray(block_cols)
    )
    xT = np.ascontiguousarray(x.T)

    in_maps = []
    for c in range(N_CORES):
        tg, og = divmod(c, OG)
        in_maps.append(
            _pack_core_inputs(
                xT[:, tg * T_SH : (tg + 1) * T_SH],
                wT[:, og * O_SH : (og + 1) * O_SH],
            )
        )

    nc = _build_nc()
    res = None
    for attempt in range(3):  # transient NRT device errors happen; retry
        try:
            res = run_bass_kernel_spmd(
                nc, in_maps, core_ids=list(range(N_CORES)), trace=TRACE
            )
            break
        except Exception:
            if attempt == 2:
                raise
            import time

            time.sleep(3)
    _last_result = res

    y = np.empty((TOKENS, OUT_F), np.float32)
    for c in range(N_CORES):
        tg, og = divmod(c, OG)
        y[tg * T_SH : (tg + 1) * T_SH, og * O_SH : (og + 1) * O_SH] = res.results[c][
            "y"
        ]
    return y
